# revision 24
# baseline (speedup 1.0000x reference)
"""Trainium2 Bass kernel for nn_Attentionlayer_84576495993011.

Full attention layer: q/k/v = x@W+b, scores = q@k^T + mask, softmax,
z = attn@v / E^0.25, out = z@Wo + bo.  B=4, S=4096, E=64, D=512.

Sharding: data-parallel over (batch, query-half) -> 8 cores, each core
computes 2048 queries x 4096 keys for one batch. Params replicated.

Key algebraic trick: scores = (x@Wq+bq)(x@Wk+bk)^T factors through the
rank-64 core M = Wq@Wk^T:
    scores[q,k] = (x@M)[q] . x[k] + u[q] + w[k] + c
with u = x@(Wq@bk), w = x@(Wk@bq), c = bq.bk -- so the big score matmul
contracts over 66 (64 + two bias-augmentation rows) instead of 512.

Softmax uses a constant shift (exp(s - 20)) instead of a row-max pass
(logits here are bounded well below fp32 exp overflow), and the row sums
come for free from the ScalarEngine activation accumulator.  attn@v is
computed transposed (zT = v^T @ attn^T) using hardware DMA-transpose of
the bf16 exp matrix; score arithmetic stays fp32 (float32r matmuls).

Walrus constraint: fp32/fp32r matmuls lower to a combined LDWEIGHTS
struct that accepts only ONE sync wait, so (a) every fp32r-consumed
constant arrives in a single packed DMA (one semaphore lane), and (b)
the mask add writes to SBUF (not in-place psum) so each score-psum slot
is released by exactly one engine.
"""

import sys

for _p in ("/opt/trn_rl_repo",):
    if _p not in sys.path:
        sys.path.insert(0, _p)

import numpy as np
import ml_dtypes

B, S, E, H = 4, 4096, 64, 8
D = E * H  # 512
SQ = S // 2  # queries per core
NCORES = 8
NQB = SQ // 128  # 16 query blocks per core
NKS = S // 512  # 8 key slabs (score matmul free dim)
NKB = S // 128  # 32 key chunks (zT contraction)
NSB = NQB // 2  # 8 query superblocks (256 queries each)
CSHIFT = 20.0  # constant logit shift (replaces row-max subtraction)
RSCALE = float(E ** -0.25)

# single packed constants tensor [128, PW] (fp32r bytes == fp32 bytes).
# One DMA -> one semaphore lane, because fp32r matmuls accept only 1 wait.
_C_XT = 0            # cols [0, S): xT (rows 0:64; rows 64/65 filled on device)
_C_XTQ = S           # cols [S, S+SQ): xTq
_C_P1A = S + SQ      # [.., +2): p1a
_C_P2A = S + SQ + 2  # [.., +2): p2a
_C_M = S + SQ + 4    # [.., +64): M
_C_WV = S + SQ + 68  # [.., +512): Wv_aug (rows 0:65)
_C_BO = S + SQ + 68 + D   # [.., +64): bo_rep (rows 0:128)
_C_BU = _C_BO + 64   # [.., +1): bias_u rows 0:2
_C_BW = _C_BU + 1    # [.., +1): bias_w rows 0:2
_C_NC = _C_BW + 1    # [.., +1): -CSHIFT all rows
PW = _C_NC + 1

_built = {}


def _build_nc(variant=""):
    """Build the per-core Bass program (same program on all 8 cores).

    variant: comma-separated debug switches for cost-model A/B runs
    ("nomask" drops the mask DMA+add, "notrans" drops the exp transposes,
    "nozt" drops the attn@v/output stage, "nosc" drops scores+exp).
    Production = "".
    """
    import os
    variant = variant or os.environ.get("KVAR", "")
    nomask = "nomask" in variant
    notrans = "notrans" in variant
    nozt = "nozt" in variant
    nosc = "nosc" in variant
    import concourse.bass as bass
    import concourse.mybir as mybir
    import concourse.tile as tile
    from concourse import bacc
    from concourse.bass import ts, ds
    from contextlib import ExitStack

    f32 = mybir.dt.float32
    f32r = mybir.dt.float32r
    bf16 = mybir.dt.bfloat16
    Exp = mybir.ActivationFunctionType.Exp
    Ident = mybir.ActivationFunctionType.Identity
    ADD = mybir.AluOpType.add
    MULT = mybir.AluOpType.mult
    AX = mybir.AxisListType.X

    nc = bacc.Bacc(trn_type="TRN2", debug=False)

    pack_r = nc.dram_tensor("pack_r", [128, PW], f32r,
                            kind="ExternalInput").ap()
    Wo_n = nc.dram_tensor("Wo_n", [128, 4 * E], bf16, kind="ExternalInput").ap()
    mask_s = nc.dram_tensor("mask_s", [SQ, S], f32, kind="ExternalInput").ap()
    out_q = nc.dram_tensor("out_q", [SQ, E], f32, kind="ExternalOutput").ap()

    tune = dict(maskbufs=2, mskdbufs=3, expbufs=2, expTbufs=2, scbufs=3)
    for kv in os.environ.get("KTUNE", "").split(","):
        if "=" in kv:
            k, v = kv.split("=")
            tune[k] = int(v)

    with tile.TileContext(nc) as tc, ExitStack() as ctx:
        const = ctx.enter_context(tc.tile_pool(name="const", bufs=1))
        maskp = ctx.enter_context(tc.tile_pool(name="maskp", bufs=tune["maskbufs"]))
        mskdp = ctx.enter_context(tc.tile_pool(name="mskdp", bufs=tune["mskdbufs"]))
        expp = ctx.enter_context(tc.tile_pool(name="expp", bufs=tune["expbufs"]))
        expTp = ctx.enter_context(tc.tile_pool(name="expTp", bufs=tune["expTbufs"]))
        ztp = ctx.enter_context(tc.tile_pool(name="ztp", bufs=2))
        outp = ctx.enter_context(tc.tile_pool(name="outp", bufs=2))
        sumsp = ctx.enter_context(tc.tile_pool(name="sumsp", bufs=4))
        ps_sc = ctx.enter_context(
            tc.tile_pool(name="ps_sc", bufs=tune["scbufs"], space="PSUM"))
        ps_zt = ctx.enter_context(tc.tile_pool(name="ps_zt", bufs=4, space="PSUM"))
        ps_o = ctx.enter_context(tc.tile_pool(name="ps_o", bufs=1, space="PSUM"))

        # ---------- stage 0: constants and projections ----------
        pk = const.tile([128, PW], f32r)      # single packed constants tile
        yTa = const.tile([E + 2, SQ], f32r)   # rows 0:64 yT | 64 u | 65 ones
        v_sb = const.tile([128, NKB * D], bf16)  # v[kb*128+p, d] at [p, kb*D+d]
        dumm = const.tile([1, 4], f32)        # dep-absorber scratch

        nc.sync.dma_start(pk[:], pack_r)
        Wo_sb = const.tile([128, 4 * E], bf16)
        nc.sync.dma_start(Wo_sb[:], Wo_n)

        xTw1 = pk[0:E + 2, _C_XT:_C_XT + S]   # [66, S]
        xTq_sb = pk[0:E, _C_XTQ:_C_XTQ + SQ]
        p1a_sb = pk[0:E, _C_P1A:_C_P1A + 2]
        p2a_sb = pk[0:E, _C_P2A:_C_P2A + 2]
        M_sb = pk[0:E, _C_M:_C_M + E]
        Wv_sb = pk[0:E + 1, _C_WV:_C_WV + D]
        bo_sb = pk[:, _C_BO:_C_BO + E].bitcast(f32)
        bu_sb = pk[0:2, _C_BU:_C_BU + 1].bitcast(f32)
        bw_sb = pk[0:2, _C_BW:_C_BW + 1].bitcast(f32)
        negC = pk[:, _C_NC:_C_NC + 1].bitcast(f32)

        # xTw1 rows 64/65 = [ones, w]: w = x @ p2 + c
        for i in range(NKS):
            ps = ps_sc.tile([128, 512], f32, tag="ps")
            nc.tensor.matmul(ps[0:2, :], lhsT=p2a_sb,
                             rhs=xTw1[0:E, ts(i, 512)], start=True, stop=True)
            nc.scalar.activation(xTw1[E:E + 2, ts(i, 512)], ps[0:2, :],
                                 Ident, bias=bw_sb, scale=1.0)

        # yTa rows 0:64 = yT = M^T x^T ; rows 64/65 = [u, ones], u = x @ p1
        for i in range(SQ // 512):
            ps = ps_sc.tile([128, 512], f32, tag="ps")
            nc.tensor.matmul(ps[0:E, :], lhsT=M_sb,
                             rhs=xTq_sb[:, ts(i, 512)], start=True, stop=True)
            nc.scalar.copy(yTa[0:E, ts(i, 512)], ps[0:E, :])
            ps2 = ps_sc.tile([128, 512], f32, tag="ps")
            nc.tensor.matmul(ps2[0:2, :], lhsT=p1a_sb,
                             rhs=xTq_sb[:, ts(i, 512)], start=True, stop=True)
            nc.scalar.activation(yTa[E:E + 2, ts(i, 512)], ps2[0:2, :],
                                 Ident, bias=bu_sb, scale=1.0)

        # v = x @ Wv + bv  (natural layout, bf16)
        for kb in range(NKB):
            ps = ps_sc.tile([128, 512], f32, tag="ps")
            nc.tensor.matmul(ps, lhsT=xTw1[0:E + 1, ts(kb, 128)],
                             rhs=Wv_sb, start=True, stop=True)
            nc.scalar.copy(v_sb[:, ts(kb, D)], ps)

        # ---------- main loop (staggered: scores(sb) then zT(sb-1)) ----------
        prev = None
        for sb in range(NSB + 1):
            cur = None
            if sb < NSB:
                expT_t = expTp.tile([128, NKB, 256], bf16)
                sums_sb = []
                for j in range(2):
                    qb = sb * 2 + j
                    if not nomask:
                        mk = maskp.tile([128, S], f32)
                        nc.sync.dma_start(mk, mask_s[ts(qb, 128), :])
                        # absorb the mask DMA-lane wait on DVE so the adds
                        # stay within the 2-wait instruction limit
                        nc.vector.tensor_copy(dumm[0:1, 0:1], mk[0:1, 0:1])
                    ex = expp.tile([128, S], bf16)
                    st = sumsp.tile([128, 12], f32)
                    for i in range(NKS):
                        if nosc:
                            continue
                        ps = ps_sc.tile([128, 512], f32, tag="ps")
                        nc.tensor.matmul(ps, lhsT=yTa[:, ts(qb, 128)],
                                         rhs=xTw1[:, ts(i, 512)],
                                         start=True, stop=True)
                        if nomask:
                            nc.scalar.activation(ex[:, ts(i, 512)], ps, Exp,
                                                 bias=negC, scale=1.0,
                                                 accum_out=st[:, i:i + 1])
                        else:
                            # masked scores to SBUF: frees the psum slot via
                            # DVE alone (fp32r matmul allows only one wait)
                            mself = mskdp.tile([128, 512], f32)
                            nc.vector.tensor_add(mself, ps, mk[:, ts(i, 512)])
                            nc.scalar.activation(ex[:, ts(i, 512)], mself, Exp,
                                                 bias=negC, scale=1.0,
                                                 accum_out=st[:, i:i + 1])
                    if not nosc:
                        nc.vector.tensor_reduce(st[:, 8:9], st[:, 0:8],
                                                axis=AX, op=ADD)
                        nc.vector.reciprocal(st[:, 9:10], st[:, 8:9])
                    if not notrans:
                        # bf16 transpose: ex [128q, 4096k] -> expT[128k, kb, q]
                        nc.scalar.dma_start(expT_t[:, :, ds(j * 128, 128)], ex,
                                            transpose=True)
                    sums_sb.append(st)
                cur = (expT_t, sums_sb, sb)
            if prev is not None and not nozt:
                expT_t, sums_sb, psb = prev
                zps = [ps_zt.tile([128, 256], f32, tag="zt", name=f"zps{dc}")
                       for dc in range(4)]
                for kb in range(NKB):
                    for dc in range(4):
                        nc.tensor.matmul(
                            zps[dc],
                            lhsT=v_sb[:, ds(kb * D + dc * 128, 128)],
                            rhs=expT_t[:, kb, :],
                            start=(kb == 0), stop=(kb == NKB - 1))
                zt = ztp.tile([128, 4, 256], bf16)
                for dc in range(4):
                    nc.scalar.copy(zt[:, dc, :], zps[dc])
                ot = outp.tile([128, 2, E], f32)
                for j in range(2):
                    po = ps_o.tile([128, E], f32, tag="po")
                    for dc in range(4):
                        nc.tensor.matmul(po, lhsT=zt[:, dc, ds(j * 128, 128)],
                                         rhs=Wo_sb[:, ts(dc, E)],
                                         start=(dc == 0), stop=(dc == 3))
                    st = sums_sb[j]
                    # out = z_unnorm * (1/sums) * (Wo pre-scaled E^-.25) + bo
                    nc.vector.scalar_tensor_tensor(ot[:, j, :], po, st[:, 9:10],
                                                   bo_sb, op0=MULT, op1=ADD)
                nc.sync.dma_start(
                    out_q[ds(psb * 256, 256), :].rearrange(
                        "(j p) e -> p j e", p=128),
                    ot)
            prev = cur

    nc.compile()
    return nc


def _host_prep(inputs):
    """Host-side weight folding (tiny, O(E*D)) and per-core input slicing."""
    x = np.ascontiguousarray(np.asarray(inputs["x"], dtype=np.float32))
    mask = np.asarray(inputs["mask"], dtype=np.float32)
    Wq = np.asarray(inputs["Wq"], dtype=np.float32)
    bq = np.asarray(inputs["bq"], dtype=np.float32)
    Wk = np.asarray(inputs["Wk"], dtype=np.float32)
    bk = np.asarray(inputs["bk"], dtype=np.float32)
    Wv = np.asarray(inputs["Wv"], dtype=np.float32)
    bv = np.asarray(inputs["bv"], dtype=np.float32)
    Wo = np.asarray(inputs["Wo"], dtype=np.float32)
    bo = np.asarray(inputs["bo"], dtype=np.float32)

    # packed constants (shared part)
    base = np.zeros((128, PW), np.float32)
    base[0:E, _C_P1A] = Wq @ bk
    base[0:E, _C_P2A + 1] = Wk @ bq
    base[0:E, _C_M:_C_M + E] = Wq @ Wk.T
    base[0:E + 1, _C_WV:_C_WV + D] = np.vstack([Wv, bv[None, :]])
    base[:, _C_BO:_C_BO + E] = bo[None, :]
    base[0:2, _C_BU] = [0.0, 1.0]               # bias_u rows 64/65 of yTa
    base[0:2, _C_BW] = [1.0, float(bq @ bk)]    # bias_w rows 64/65 of xTw1
    base[:, _C_NC] = -CSHIFT

    # Wo chunked to [128, 4*64]: Wo_n[p, dc*64+e] = Wo[dc*128+p, e] * E^-0.25
    Wo_n = np.ascontiguousarray(
        (Wo * RSCALE).reshape(4, 128, E).transpose(1, 0, 2).reshape(128, 4 * E)
    ).astype(ml_dtypes.bfloat16)

    in_maps = []
    for core in range(NCORES):
        b, h = core // 2, core % 2
        q0 = h * SQ
        pack_r = base.copy()
        pack_r[0:E, _C_XT:_C_XT + S] = x[b].T
        pack_r[0:E, _C_XTQ:_C_XTQ + SQ] = x[b, q0:q0 + SQ].T
        in_maps.append({
            "pack_r": pack_r,
            "Wo_n": Wo_n,
            "mask_s": np.ascontiguousarray(mask[b, q0:q0 + SQ]),
        })
    return in_maps


def kernel(**inputs):
    from concourse.bass_utils import run_bass_kernel_spmd

    if "nc" not in _built:
        _built["nc"] = _build_nc()
    nc = _built["nc"]

    in_maps = _host_prep(inputs)
    trace = bool(int(__import__("os").environ.get("KERNEL_TRACE", "0")))
    res = run_bass_kernel_spmd(nc, in_maps, core_ids=list(range(NCORES)),
                               trace=trace)
    _built["last_results"] = res

    out = np.zeros((B, S, E), dtype=np.float32)
    for core in range(NCORES):
        b, h = core // 2, core % 2
        out[b, h * SQ:(h + 1) * SQ] = res.results[core]["out_q"]
    return out


# revision 43
# speedup vs baseline: 22572.4092x; 22572.4092x over previous
"""Trainium2 Bass kernel for nn_Attentionlayer_84576495993011.

Full attention layer: q/k/v = x@W+b, scores = q@k^T + mask, softmax,
z = attn@v / E^0.25, out = z@Wo + bo.  B=4, S=4096, E=64, D=512.

Sharding: data-parallel over (batch, query-half) -> 8 cores, each core
computes 2048 queries x 4096 keys for one batch. Params replicated.

Key algebraic trick: scores = (x@Wq+bq)(x@Wk+bk)^T factors through the
rank-64 core M = Wq@Wk^T:
    scores[q,k] = (x@M)[q] . x[k] + u[q] + w[k] + c
with u = x@(Wq@bk), w = x@(Wk@bq), c = bq.bk -- so the big score matmul
contracts over 66 (64 + two bias-augmentation rows) instead of 512.

Softmax uses a constant shift (exp(s - 20)) instead of a row-max pass
(logits here are bounded well below fp32 exp overflow), and the row sums
come for free from the ScalarEngine activation accumulator.  attn@v is
computed transposed (zT = v^T @ attn^T) using hardware DMA-transpose of
the bf16 exp matrix; score arithmetic stays fp32 (float32r matmuls).

Walrus constraint: fp32/fp32r matmuls lower to a combined LDWEIGHTS
struct that accepts only ONE sync wait, so (a) every fp32r-consumed
constant arrives in a single packed DMA (one semaphore lane), and (b)
the mask add writes to SBUF (not in-place psum) so each score-psum slot
is released by exactly one engine.
"""

import sys

for _p in ("/opt/trn_rl_repo",):
    if _p not in sys.path:
        sys.path.insert(0, _p)

import numpy as np
import ml_dtypes

B, S, E, H = 4, 4096, 64, 8
D = E * H  # 512
SQ = S // 2  # queries per core
NCORES = 8
NQB = SQ // 128  # 16 query blocks per core
NKS = S // 512  # 8 key slabs (score matmul free dim)
NKB = S // 128  # 32 key chunks (zT contraction)
NSB = NQB // 2  # 8 query superblocks (256 queries each)
CSHIFT = 20.0  # constant logit shift (replaces row-max subtraction)
RSCALE = float(E ** -0.25)

# single packed constants tensor [128, PW] (fp32r bytes == fp32 bytes).
# One DMA -> one semaphore lane, because fp32r matmuls accept only 1 wait.
_C_XT = 0            # cols [0, S): xT (rows 0:64; rows 64/65 filled on device)
_C_XTQ = S           # cols [S, S+SQ): xTq
_C_P1A = S + SQ      # [.., +2): p1a
_C_P2A = S + SQ + 2  # [.., +2): p2a
_C_M = S + SQ + 4    # [.., +64): M
_C_WV = S + SQ + 68  # [.., +512): Wv_aug (rows 0:65)
_C_BO = S + SQ + 68 + D   # [.., +64): bo_rep (rows 0:128)
_C_BU = _C_BO + 64   # [.., +1): bias_u rows 0:2
_C_BW = _C_BU + 1    # [.., +1): bias_w rows 0:2
_C_NC = _C_BW + 1    # [.., +1): -CSHIFT all rows
_C_ONE = _C_NC + 1   # [.., +1): 1.0 all rows (transposed-layout helpers)
PW = _C_ONE + 1

NQSB = 4  # transposed layout: 512-query superblocks per core

_built = {}
KLAYOUT = "t"  # production layout: transposed scores (no exp transposes)


def _build_nc(variant=""):
    """Build the per-core Bass program (same program on all 8 cores).

    variant: comma-separated debug switches for cost-model A/B runs
    ("nomask" drops the mask DMA+add, "notrans" drops the exp transposes,
    "nozt" drops the attn@v/output stage, "nosc" drops scores+exp).
    Production = "".
    """
    import os
    variant = variant or os.environ.get("KVAR", "")
    nomask = "nomask" in variant
    notrans = "notrans" in variant
    nozt = "nozt" in variant
    nosc = "nosc" in variant
    reps = 1  # "repN" wraps the main loop in a hardware For_i loop (timing)
    for tok in variant.split(","):
        if tok.startswith("rep"):
            reps = int(tok[3:])
    tlayout = "t" in variant.split(",")  # transposed-scores layout
    import concourse.bass as bass
    import concourse.mybir as mybir
    import concourse.tile as tile
    from concourse import bacc
    from concourse.bass import ts, ds
    from contextlib import ExitStack

    f32 = mybir.dt.float32
    f32r = mybir.dt.float32r
    bf16 = mybir.dt.bfloat16
    Exp = mybir.ActivationFunctionType.Exp
    Ident = mybir.ActivationFunctionType.Identity
    ADD = mybir.AluOpType.add
    MULT = mybir.AluOpType.mult
    AX = mybir.AxisListType.X

    nc = bacc.Bacc(trn_type="TRN2", debug=False)

    pack_r = nc.dram_tensor("pack_r", [128, PW], f32r,
                            kind="ExternalInput").ap()
    Wo_n = nc.dram_tensor("Wo_n", [128, 4 * E + 1], bf16,
                          kind="ExternalInput").ap()
    if tlayout:
        # host-transposed mask, packed per (qsb, 4-key-chunk group)
        mask_s = nc.dram_tensor("mask_s", [NQSB, 8, 128, 4 * 512], f32,
                                kind="ExternalInput").ap()
    else:
        mask_s = nc.dram_tensor("mask_s", [SQ, S], f32,
                                kind="ExternalInput").ap()
    out_q = nc.dram_tensor("out_q", [SQ, E], f32, kind="ExternalOutput").ap()

    tune = dict(maskbufs=3 if tlayout else 2, mskdbufs=3, expbufs=2,
                expTbufs=2, scbufs=2 if tlayout else 3)
    for kv in os.environ.get("KTUNE", "").split(","):
        if "=" in kv:
            k, v = kv.split("=")
            tune[k] = int(v)

    with tile.TileContext(nc) as tc, ExitStack() as ctx:
        const = ctx.enter_context(tc.tile_pool(name="const", bufs=1))
        maskp = ctx.enter_context(tc.tile_pool(name="maskp", bufs=tune["maskbufs"]))
        mskdp = ctx.enter_context(tc.tile_pool(name="mskdp", bufs=tune["mskdbufs"]))
        expp = ctx.enter_context(tc.tile_pool(name="expp", bufs=tune["expbufs"]))
        expTp = ctx.enter_context(tc.tile_pool(name="expTp", bufs=tune["expTbufs"]))
        ztp = ctx.enter_context(tc.tile_pool(name="ztp", bufs=2))
        outp = ctx.enter_context(tc.tile_pool(name="outp", bufs=2))
        sumsp = ctx.enter_context(tc.tile_pool(name="sumsp", bufs=4))
        ps_sc = ctx.enter_context(
            tc.tile_pool(name="ps_sc", bufs=tune["scbufs"], space="PSUM"))
        ps_zt = ctx.enter_context(tc.tile_pool(name="ps_zt", bufs=4, space="PSUM"))
        ps_o = ctx.enter_context(tc.tile_pool(name="ps_o", bufs=1, space="PSUM"))
        ps_su = (ctx.enter_context(tc.tile_pool(name="ps_su", bufs=1,
                                                space="PSUM"))
                 if tlayout else None)

        # ---------- stage 0: constants and projections ----------
        pk = const.tile([128, PW], f32r)      # single packed constants tile
        yTa = const.tile([E + 2, SQ], f32r)   # rows 0:64 yT | 64 u | 65 ones
        v_sb = const.tile([128, NKB * D], bf16)  # v[kb*128+p, d] at [p, kb*D+d]
        dumm = const.tile([1, 4], f32)        # dep-absorber scratch

        nc.sync.dma_start(pk[:], pack_r)
        Wo_sb = const.tile([128, 4 * E + 1], bf16)
        nc.sync.dma_start(Wo_sb[:], Wo_n)

        xTw1 = pk[0:E + 2, _C_XT:_C_XT + S]   # [66, S]
        xTq_sb = pk[0:E, _C_XTQ:_C_XTQ + SQ]
        p1a_sb = pk[0:E, _C_P1A:_C_P1A + 2]
        p2a_sb = pk[0:E, _C_P2A:_C_P2A + 2]
        M_sb = pk[0:E, _C_M:_C_M + E]
        Wv_sb = pk[0:E + 1, _C_WV:_C_WV + D]
        bo_sb = pk[:, _C_BO:_C_BO + E].bitcast(f32)
        bu_sb = pk[0:2, _C_BU:_C_BU + 1].bitcast(f32)
        bw_sb = pk[0:2, _C_BW:_C_BW + 1].bitcast(f32)
        negC = pk[:, _C_NC:_C_NC + 1].bitcast(f32)

        # xTw1 rows 64/65 = [ones, w]: w = x @ p2 + c
        for i in range(NKS):
            ps = ps_sc.tile([128, 512], f32, tag="ps")
            nc.tensor.matmul(ps[0:2, :], lhsT=p2a_sb,
                             rhs=xTw1[0:E, ts(i, 512)], start=True, stop=True)
            nc.scalar.activation(xTw1[E:E + 2, ts(i, 512)], ps[0:2, :],
                                 Ident, bias=bw_sb, scale=1.0)

        # yTa rows 0:64 = yT = M^T x^T ; rows 64/65 = [u, ones], u = x @ p1
        for i in range(SQ // 512):
            ps = ps_sc.tile([128, 512], f32, tag="ps")
            nc.tensor.matmul(ps[0:E, :], lhsT=M_sb,
                             rhs=xTq_sb[:, ts(i, 512)], start=True, stop=True)
            nc.scalar.copy(yTa[0:E, ts(i, 512)], ps[0:E, :])
            ps2 = ps_sc.tile([128, 512], f32, tag="ps")
            nc.tensor.matmul(ps2[0:2, :], lhsT=p1a_sb,
                             rhs=xTq_sb[:, ts(i, 512)], start=True, stop=True)
            nc.scalar.activation(yTa[E:E + 2, ts(i, 512)], ps2[0:2, :],
                                 Ident, bias=bu_sb, scale=1.0)

        # v = x @ Wv + bv  (natural layout, bf16)
        for kb in range(NKB):
            ps = ps_sc.tile([128, 512], f32, tag="ps")
            nc.tensor.matmul(ps, lhsT=xTw1[0:E + 1, ts(kb, 128)],
                             rhs=Wv_sb, start=True, stop=True)
            nc.scalar.copy(v_sb[:, ts(kb, D)], ps)

        # ------- transposed-scores main loop: exp born in [k, q] layout ------
        def main_body_t(_iv=None):
            ones_bf = Wo_sb[:, 4 * E:4 * E + 1]       # bf16 ones column
            ones_r = pk[0:1, _C_ONE:_C_ONE + 1]       # f32r scalar one
            prev = None
            for qsb in range(NQSB + 1):
                cur = None
                if qsb < NQSB:
                    expT_t = expTp.tile([128, NKB, 512], bf16)
                    for g in range(8):
                        mk = maskp.tile([128, 4, 512], f32)
                        nc.sync.dma_start(mk, mask_s[qsb, g].rearrange(
                            "p (l q) -> p l q", l=4))
                        nc.vector.tensor_copy(dumm[0:1, 0:1], mk[0:1, 0, 0:1])
                        for kbl in range(4):
                            kb = g * 4 + kbl
                            ps = ps_sc.tile([128, 512], f32, tag="ps")
                            nc.tensor.matmul(ps, lhsT=xTw1[:, ts(kb, 128)],
                                             rhs=yTa[:, ts(qsb, 512)],
                                             start=True, stop=True)
                            msk = mskdp.tile([128, 512], f32)
                            nc.vector.tensor_add(msk, ps, mk[:, kbl, :])
                            nc.scalar.activation(expT_t[:, kb, :], msk, Exp,
                                                 bias=negC, scale=1.0)
    # softmax denominators, directly in per-partition column form:
                    # su[q, 0] = sum_k exp[k, q] via expT^T @ ones (bf16)
                    sums_sb = []
                    for qq in range(4):
                        su = ps_su.tile([128, 1], f32, tag="su")
                        for kb in range(NKB):
                            nc.tensor.matmul(
                                su, lhsT=expT_t[:, kb, ds(qq * 128, 128)],
                                rhs=ones_bf,
                                start=(kb == 0), stop=(kb == NKB - 1))
                        st = sumsp.tile([128, 1], f32, name="st")
                        nc.vector.reciprocal(st[:], su)
                        sums_sb.append(st)
                    cur = (expT_t, sums_sb, qsb)
                if prev is not None and not nozt:
                    expT_t, sums_sb, pq = prev
                    zps = [ps_zt.tile([128, 512], f32, tag="zt",
                                      name=f"zps{dc}") for dc in range(4)]
                    for kb in range(NKB):
                        for dc in range(4):
                            nc.tensor.matmul(
                                zps[dc],
                                lhsT=v_sb[:, ds(kb * D + dc * 128, 128)],
                                rhs=expT_t[:, kb, :],
                                start=(kb == 0), stop=(kb == NKB - 1))
                    zt = ztp.tile([128, 4, 512], bf16)
                    for dc in range(4):
                        nc.scalar.copy(zt[:, dc, :], zps[dc])
                    ot = outp.tile([128, 4, E], f32)
                    for qq in range(4):
                        po = ps_o.tile([128, E], f32, tag="po")
                        for dc in range(4):
                            nc.tensor.matmul(po,
                                             lhsT=zt[:, dc, ds(qq * 128, 128)],
                                             rhs=Wo_sb[:, ts(dc, E)],
                                             start=(dc == 0), stop=(dc == 3))
                        nc.vector.scalar_tensor_tensor(ot[:, qq, :], po,
                                                       sums_sb[qq][:], bo_sb,
                                                       op0=MULT, op1=ADD)
                    nc.sync.dma_start(
                        out_q[ds(pq * 512, 512), :].rearrange(
                            "(j p) e -> p j e", p=128),
                        ot)
                prev = cur

        # ---------- main loop (staggered: scores(sb) then zT(sb-1)) ----------
        def main_body(_iv=None):
          if tlayout:
              main_body_t(_iv)
              return
          prev = None
          for sb in range(NSB + 1):
            cur = None
            if sb < NSB:
                expT_t = expTp.tile([128, NKB, 256], bf16)
                sums_sb = []
                for j in range(2):
                    qb = sb * 2 + j
                    if not nomask:
                        mk = maskp.tile([128, S], f32)
                        nc.sync.dma_start(mk, mask_s[ts(qb, 128), :])
                        # absorb the mask DMA-lane wait on DVE so the adds
                        # stay within the 2-wait instruction limit
                        nc.vector.tensor_copy(dumm[0:1, 0:1], mk[0:1, 0:1])
                    ex = expp.tile([128, S], bf16)
                    st = sumsp.tile([128, 12], f32)
                    for i in range(NKS):
                        if nosc:
                            continue
                        ps = ps_sc.tile([128, 512], f32, tag="ps")
                        nc.tensor.matmul(ps, lhsT=yTa[:, ts(qb, 128)],
                                         rhs=xTw1[:, ts(i, 512)],
                                         start=True, stop=True)
                        if nomask:
                            nc.scalar.activation(ex[:, ts(i, 512)], ps, Exp,
                                                 bias=negC, scale=1.0,
                                                 accum_out=st[:, i:i + 1])
                        else:
                            # masked scores to SBUF: frees the psum slot via
                            # DVE alone (fp32r matmul allows only one wait)
                            mself = mskdp.tile([128, 512], f32)
                            nc.vector.tensor_add(mself, ps, mk[:, ts(i, 512)])
                            nc.scalar.activation(ex[:, ts(i, 512)], mself, Exp,
                                                 bias=negC, scale=1.0,
                                                 accum_out=st[:, i:i + 1])
                    if not nosc:
                        nc.vector.tensor_reduce(st[:, 8:9], st[:, 0:8],
                                                axis=AX, op=ADD)
                        nc.vector.reciprocal(st[:, 9:10], st[:, 8:9])
                    if not notrans:
                        # bf16 transpose: ex [128q, 4096k] -> expT[128k, kb, q]
                        nc.scalar.dma_start(expT_t[:, :, ds(j * 128, 128)], ex,
                                            transpose=True)
                    sums_sb.append(st)
                cur = (expT_t, sums_sb, sb)
            if prev is not None and not nozt:
                expT_t, sums_sb, psb = prev
                zps = [ps_zt.tile([128, 256], f32, tag="zt", name=f"zps{dc}")
                       for dc in range(4)]
                for kb in range(NKB):
                    for dc in range(4):
                        nc.tensor.matmul(
                            zps[dc],
                            lhsT=v_sb[:, ds(kb * D + dc * 128, 128)],
                            rhs=expT_t[:, kb, :],
                            start=(kb == 0), stop=(kb == NKB - 1))
                zt = ztp.tile([128, 4, 256], bf16)
                for dc in range(4):
                    nc.scalar.copy(zt[:, dc, :], zps[dc])
                ot = outp.tile([128, 2, E], f32)
                for j in range(2):
                    po = ps_o.tile([128, E], f32, tag="po")
                    for dc in range(4):
                        nc.tensor.matmul(po, lhsT=zt[:, dc, ds(j * 128, 128)],
                                         rhs=Wo_sb[:, ts(dc, E)],
                                         start=(dc == 0), stop=(dc == 3))
                    st = sums_sb[j]
                    # out = z_unnorm * (1/sums) * (Wo pre-scaled E^-.25) + bo
                    nc.vector.scalar_tensor_tensor(ot[:, j, :], po, st[:, 9:10],
                                                   bo_sb, op0=MULT, op1=ADD)
                nc.sync.dma_start(
                    out_q[ds(psb * 256, 256), :].rearrange(
                        "(j p) e -> p j e", p=128),
                    ot)
            prev = cur

        if reps == 1:
            main_body()
        else:
            with tc.For_i(0, reps, 1):
                main_body()

    nc.compile()
    return nc


def _host_prep(inputs, tlayout=None):
    """Host-side weight folding (tiny, O(E*D)) and per-core input slicing."""
    if tlayout is None:
        tlayout = KLAYOUT == "t"
    x = np.ascontiguousarray(np.asarray(inputs["x"], dtype=np.float32))
    mask = np.asarray(inputs["mask"], dtype=np.float32)
    Wq = np.asarray(inputs["Wq"], dtype=np.float32)
    bq = np.asarray(inputs["bq"], dtype=np.float32)
    Wk = np.asarray(inputs["Wk"], dtype=np.float32)
    bk = np.asarray(inputs["bk"], dtype=np.float32)
    Wv = np.asarray(inputs["Wv"], dtype=np.float32)
    bv = np.asarray(inputs["bv"], dtype=np.float32)
    Wo = np.asarray(inputs["Wo"], dtype=np.float32)
    bo = np.asarray(inputs["bo"], dtype=np.float32)

    # packed constants (shared part)
    base = np.zeros((128, PW), np.float32)
    base[0:E, _C_P1A] = Wq @ bk
    base[0:E, _C_P2A + 1] = Wk @ bq
    base[0:E, _C_M:_C_M + E] = Wq @ Wk.T
    base[0:E + 1, _C_WV:_C_WV + D] = np.vstack([Wv, bv[None, :]])
    base[:, _C_BO:_C_BO + E] = bo[None, :]
    base[0:2, _C_BU] = [0.0, 1.0]               # bias_u rows 64/65 of yTa
    base[0:2, _C_BW] = [1.0, float(bq @ bk)]    # bias_w rows 64/65 of xTw1
    base[:, _C_NC] = -CSHIFT
    base[:, _C_ONE] = 1.0

    # Wo chunked to [128, 4*64]: Wo_n[p, dc*64+e] = Wo[dc*128+p, e] * E^-0.25
    # plus a bf16 ones column (transposed-layout softmax row sums)
    Wo_n = np.ones((128, 4 * E + 1), np.float32)
    Wo_n[:, 0:4 * E] = (Wo * RSCALE).reshape(4, 128, E).transpose(
        1, 0, 2).reshape(128, 4 * E)
    Wo_n = np.ascontiguousarray(Wo_n).astype(ml_dtypes.bfloat16)

    in_maps = []
    for core in range(NCORES):
        b, h = core // 2, core % 2
        q0 = h * SQ
        pack_r = base.copy()
        pack_r[0:E, _C_XT:_C_XT + S] = x[b].T
        pack_r[0:E, _C_XTQ:_C_XTQ + SQ] = x[b, q0:q0 + SQ].T
        if tlayout:
            # [k, q] tiles packed per (qsb, 4-key-chunk group)
            mt = np.ascontiguousarray(mask[b, q0:q0 + SQ].T)  # [S, SQ]
            mp = mt.reshape(8, 4, 128, NQSB, 512).transpose(3, 0, 2, 1, 4)
            ms = np.ascontiguousarray(mp.reshape(NQSB, 8, 128, 4 * 512))
        else:
            ms = np.ascontiguousarray(mask[b, q0:q0 + SQ])
        in_maps.append({
            "pack_r": pack_r,
            "Wo_n": Wo_n,
            "mask_s": ms,
        })
    return in_maps


def kernel(**inputs):
    from concourse.bass_utils import run_bass_kernel_spmd

    if "nc" not in _built:
        _built["nc"] = _build_nc(variant=KLAYOUT if KLAYOUT == "t" else "none")
    nc = _built["nc"]

    in_maps = _host_prep(inputs, tlayout=(KLAYOUT == "t"))
    trace = bool(int(__import__("os").environ.get("KERNEL_TRACE", "0")))
    res = run_bass_kernel_spmd(nc, in_maps, core_ids=list(range(NCORES)),
                               trace=trace)
    _built["last_results"] = res

    out = np.zeros((B, S, E), dtype=np.float32)
    for core in range(NCORES):
        b, h = core // 2, core % 2
        out[b, h * SQ:(h + 1) * SQ] = res.results[core]["out_q"]
    return out
